# revision 40
# baseline (speedup 1.0000x reference)
"""CondInst dynamic mask head on 8 Trainium2 NeuronCores (v3: all-bf16).

Math per instance i: x_i = [rel_i (2,HW); feats_b (8,HW)],
  h1 = relu(w0_i x_i + b0_i); h2 = relu(w1_i h1 + b1_i);
  out_i = sigmoid(w2_i h2 + b2_i).

rel_i is affine in the shared coords map -> folded into shared
X = [coords/128; feats] with per-instance Ahat_i and bias c0_i.

Measured TRN2 matmul behavior (this silicon):
- moving streams ~1 col/cycle @2.4GHz only when the contraction partition
  count is ~>=120; K<=64 runs at ~half rate (458ns vs 252ns per 512-col MM).
- interleaving matmuls of different K-class/dtype costs ~280ns per switch;
  bf16-K121 next to fp8-DR-K120 is free, but plain-fp8 next to bf16 is not.
- fp8 DoubleRow gives NO streaming speedup here, and its outputs must sit
  at PE column band 0 -> useless for multi-block packing.
- matmuls with <=32-wide outputs at distinct 32-aligned column positions
  overlap each other almost fully.

So: EVERY matmul is plain bf16 with K=120/121 (one shape class):
- L0 per group: stationary [121, 8n+1] (rows 0-9 = Ahat, row 10 = c0 bias
  vs the ones-row of X, col 8n emits a constant 1.0 for L1's bias row),
  moving = X padded to 121 rows.  Group q3 is padded to full height.
- L1 per group: block-diag [121, 8n] with row 120(80->padded) = b1.
- L2 per group: [120, 32] stationaries into 7 32-aligned blocks of two psum
  tiles pA/pB; the 7 matmuls are emitted adjacently per chunk so they
  overlap across column positions.
- Evacuations are pure relu (no bias operand) on Act/DVE (GPSIMD cannot
  read PSUM); sigmoid+b2 on Act from pA/pB into SBUF; batched DMA out.
"""

import os
import sys

import numpy as np

sys.path.insert(0, "/opt/trn_rl_repo")
os.environ.setdefault("MYCRO_LOCAL_CACHE", "1")

B, K, C, H, Wd = 4, 100, 8, 128, 128
HW = H * Wd
LC = HW // 2            # 8192 px per core
WCH = 512               # px per chunk
NCH = LC // WCH         # 16 chunks
NCORE = 8
GS = [15, 15, 15, 15, 15, 15, 10]      # group sizes (7 groups, 100 inst)
GOFF = [0, 15, 30, 45, 60, 75, 90]
NPAIR = 4                              # tasks per chunk: (0,1),(2,3),(4,5),(6,)
# L2 output block of group g: (tile 0=pA/1=pB, partition base)
L2BLK = [(0, 0), (0, 32), (0, 64), (0, 96), (1, 0), (1, 32), (1, 64)]

_PROGRAM = None


def _prep_inputs(seg_feat, conv_weight, ind):
    import ml_dtypes
    bf16 = ml_dtypes.bfloat16

    seg_feat = np.asarray(seg_feat, dtype=np.float32)
    conv_weight = np.asarray(conv_weight, dtype=np.float32)
    ind64 = np.asarray(ind).astype(np.int64)

    cw = conv_weight.reshape(B, -1, HW)
    params = np.take_along_axis(cw, ind64[:, None, :], axis=2)  # [B,P,K]
    params = params.transpose(0, 2, 1)                           # [B,K,P]

    w0 = params[..., 0:80].reshape(B, K, C, C + 2)
    w1 = params[..., 80:144].reshape(B, K, C, C)
    w2 = params[..., 144:152].reshape(B, K, 1, C)
    b0 = params[..., 152:160]
    b1 = params[..., 160:168]
    b2 = params[..., 168:169]

    xi = (ind64 % Wd).astype(np.float32)
    yi = (ind64 // Wd).astype(np.float32)
    loc = np.stack([xi, yi], axis=-1)
    w0r = w0[..., 0:2]
    w0f = w0[..., 2:10]
    ahat = np.concatenate([-w0r, w0f], axis=-1)                  # [B,K,8,10]
    c0 = b0 + np.einsum("bkoc,bkc->bko", w0r, loc) / 128.0       # [B,K,8]

    lin = np.arange(HW, dtype=np.float32)
    coords_x = (lin % Wd) / 128.0
    coords_y = np.floor(lin / Wd) / 128.0

    in_maps = []
    for core in range(NCORE):
        b = core // 2
        sl = slice((core % 2) * LC, (core % 2) * LC + LC)

        # ---- xq [121, LC] bf16: L0 moving, padded to the fast K-class
        xq = np.zeros((121, LC), np.float32)
        xq[0] = coords_x[sl]
        xq[1] = coords_y[sl]
        xq[2:10] = seg_feat[b].reshape(C, HW)[:, sl]
        xq[10] = 1.0                     # bias row (c0) + L1 ones source

        # ---- w0s [121, 7*128] bf16 (col 128g+8j+o; col 128g+8n -> 1.0 row)
        w0s = np.zeros((121, 7 * 128), np.float32)
        # ---- w1s [121, 7*128] bf16 (row 8n = b1 ones-row)
        w1s = np.zeros((121, 7 * 128), np.float32)
        # ---- w2s [120, 256] bf16: group g at cols 32g..32g+32
        w2s = np.zeros((120, 2, 128), np.float32)

        for g in range(7):
            n = GS[g]
            for j in range(n):
                i = GOFF[g] + j
                m = 128 * g + 8 * j
                w0s[0:10, m:m + 8] = ahat[b, i].T
                w0s[10, m:m + 8] = c0[b, i]
                w1s[8 * j:8 * j + 8, m:m + 8] = w1[b, i].T
                w1s[8 * n, m:m + 8] = b1[b, i]
                u = 32 * (g // 2) + 15 * (g % 2) + j
                w2s[8 * j:8 * j + 8, g % 2, u] = w2[b, i, 0]
            # constant-1.0 L0 output col (rides the bias row)
            w0s[10, 128 * g + 8 * n] = 1.0

        b2sa = np.zeros((128, 1), np.float32)
        for blk in range(4):
            lo, n = 30 * blk, min(30, 100 - 30 * blk)
            b2sa[32 * blk:32 * blk + n, 0] = b2[b, lo:lo + n, 0]

        in_maps.append({
            "xq": xq.astype(bf16),
            "w0s": w0s.astype(bf16),
            "w1s": w1s.astype(bf16),
            "w2s": w2s.astype(bf16),
            "b2sa": b2sa,
        })

    return in_maps, (b2, np.asarray(ind).dtype)


def build_program():
    global _PROGRAM
    if _PROGRAM is not None:
        return _PROGRAM

    import concourse.tile as tile
    from concourse import bacc, mybir

    nc = bacc.Bacc("TRN2", target_bir_lowering=False, debug=False)
    f32 = mybir.dt.float32
    bf16 = mybir.dt.bfloat16
    Relu = mybir.ActivationFunctionType.Relu
    Sigmoid = mybir.ActivationFunctionType.Sigmoid

    xq_h = nc.dram_tensor("xq", [121, LC], bf16, kind="ExternalInput")
    w0s_h = nc.dram_tensor("w0s", [121, 7 * 128], bf16, kind="ExternalInput")
    w1s_h = nc.dram_tensor("w1s", [121, 7 * 128], bf16, kind="ExternalInput")
    w2s_h = nc.dram_tensor("w2s", [120, 2, 128], bf16, kind="ExternalInput")
    b2sa_h = nc.dram_tensor("b2sa", [128, 1], f32, kind="ExternalInput")
    out_h = nc.dram_tensor("out_shard", [100, LC], f32, kind="ExternalOutput")

    with tile.TileContext(nc) as tc:
        with (
            tc.tile_pool(name="const", bufs=1) as cpool,
            tc.tile_pool(name="h1p", bufs=6) as h1pool,
            tc.tile_pool(name="h2p", bufs=10) as h2pool,
            tc.tile_pool(name="ps", bufs=1, space="PSUM") as pspool,
        ):
            xq = cpool.tile([121, LC], bf16, tag="xq")
            w0s = cpool.tile([121, 7 * 128], bf16, tag="w0s")
            w1s = cpool.tile([121, 7 * 128], bf16, tag="w1s")
            w2s = cpool.tile([120, 2, 128], bf16, tag="w2s")
            b2sa = cpool.tile([128, 1], f32, tag="b2sa")
            outba = cpool.tile([106, LC], f32, tag="outba")

            nc.gpsimd.dma_start(w0s[:], w0s_h[:])
            nc.gpsimd.dma_start(b2sa[:], b2sa_h[:])
            # chunked xq load so the first L0 only waits for the first slice
            for dq in range(8):
                qs = slice(dq * (LC // 8), (dq + 1) * (LC // 8))
                nc.gpsimd.dma_start(xq[:, qs], xq_h[:, qs])
            nc.gpsimd.dma_start(w1s[:], w1s_h[:])
            nc.gpsimd.dma_start(w2s[:], w2s_h[:])

            # PE p-state pre-warm on w0s while the xq/w1s DMAs land
            warm = pspool.tile([128, WCH], f32, tag="pa", bufs=1, name="warm")
            for i in range(48):
                nc.tensor.matmul(warm[0:32, 0:256], w0s[:, 0:32],
                                 w0s[:, 0:256], tile_position=(0, 0))

            # software pipeline over tasks s = 4*chunk + pair
            NT = NCH * NPAIR
            pw_by, h1_by, h2_by, p1_by, pa_by = {}, {}, {}, {}, {}

            # evac engine per q: 0 = scalar(Act), 1 = vector(DVE)
            EV_H1 = [1, 0, 1, 0]
            EV_H2 = [0, 1, 0, 1]

            def evac(e, out_t, in_t):
                # pure relu: out = max(in, 0)
                if e == 0:
                    nc.scalar.activation(out_t, in_t, Relu)
                else:
                    nc.vector.tensor_scalar_max(out_t, in_t, 0.0)

            for s in range(NT + 5):
                c, q = s // NPAIR, s % NPAIR

                # ---- h1 = relu(pw0) for task s-1 (pair op)
                if 0 <= s - 1 < NT:
                    q1 = (s - 1) % NPAIR
                    pw = pw_by[s - 1]
                    h1 = h1pool.tile([128, 2, WCH], bf16, tag="h1",
                                     name=f"h1_{s - 1}")
                    h1_by[s - 1] = h1
                    if q1 < 3:
                        evac(EV_H1[q1], h1[0:121, :, :], pw[0:121, :, :])
                    else:
                        evac(EV_H1[q1], h1[0:121, 0, :], pw[0:121, 0, :])

                # ---- h2 = relu(pw1) for task s-3 (per-group ops)
                if 0 <= s - 3 < NT:
                    q2 = (s - 3) % NPAIR
                    c3 = (s - 3) // NPAIR
                    h2 = h2pool.tile([128, 2, WCH], bf16, tag="h2",
                                     name=f"h2_{s - 3}")
                    h2_by[s - 3] = h2
                    for t in range(2):
                        g = 2 * q2 + t
                        if g >= 7:
                            continue
                        p1g = p1_by.pop((s - 3, t))
                        e = (EV_H2[q2] + t) % 2
                        if q2 == 3:
                            e = (1 + c3) % 2
                        evac(e, h2[0:120, t, :], p1g[0:120, :])

                # ---- L0 for task s (both groups; q3 padded to 121 rows)
                if s < NT:
                    fl = slice(c * WCH, (c + 1) * WCH)
                    pw = pspool.tile([128, 2, WCH], f32, tag="pw", bufs=2,
                                     name=f"pw_{s}")
                    pw_by[s] = pw
                    for t in range(2):
                        g = 2 * q + t
                        if g >= 7:
                            continue
                        nc.tensor.matmul(
                            pw[0:121, t, :],
                            w0s[:, 128 * g:128 * g + 121],
                            xq[:, fl],
                        )

                # ---- L1 for task s-2 (per-group psum tiles, lag 2)
                if 0 <= s - 2 < NT:
                    q1 = (s - 2) % NPAIR
                    pw_by.pop(s - 2, None)
                    h1 = h1_by.pop(s - 2)
                    for t in range(2):
                        g = 2 * q1 + t
                        if g >= 7:
                            continue
                        p1g = pspool.tile([128, WCH], f32, tag="p1", bufs=3,
                                          name=f"p1_{s - 2}_{t}")
                        p1_by[(s - 2, t)] = p1g
                        nc.tensor.matmul(
                            p1g[0:120, :],
                            w1s[0:121, 128 * g:128 * g + 120],
                            h1[0:121, t, :],
                        )

                # ---- L2: all 7 groups of chunk c2, batched; two groups
                # accumulate into each 30-wide 32-aligned block of pa
                if 0 <= s - 3 < NT and (s - 3) % NPAIR == NPAIR - 1:
                    c2 = (s - 3) // NPAIR
                    pa = pspool.tile([128, WCH], f32, tag="pa", bufs=1,
                                     name=f"pa_{c2}")
                    for g in (0, 2, 4, 6, 1, 3, 5):
                        sg, tg = 4 * c2 + g // 2, g % 2
                        h2 = h2_by[sg]
                        blk = g // 2
                        nc.tensor.matmul(
                            pa[32 * blk:32 * blk + 30, :],
                            w2s[:, g % 2, 32 * blk:32 * blk + 30],
                            h2[0:120, tg, :],
                            start=(g % 2 == 0),
                            stop=(g % 2 == 1 or g == 6),
                            skip_group_check=True,
                            tile_position=(0, 32 * blk),
                        )
                    for g in range(7):
                        h2_by.pop(4 * c2 + g // 2, None)
                    flc = slice(c2 * WCH, (c2 + 1) * WCH)
                    nc.scalar.activation(outba[0:106, flc], pa[0:106, :],
                                         Sigmoid, bias=b2sa[0:106, :],
                                         scale=1.0)
                    if c2 == NCH - 2:
                        # last-but-one chunk ships alone so only chunk 15's
                        # DMAs remain in the tail
                        for blk in range(4):
                            lo, n = 30 * blk, min(30, 100 - 30 * blk)
                            nc.gpsimd.dma_start(
                                out_h[lo:lo + n, flc],
                                outba[32 * blk:32 * blk + n, flc])
                    elif c2 == NCH - 1:
                        # tail DMAs: spread descriptor generation across
                        # engine queues that are idle by now
                        engs = [nc.gpsimd, nc.scalar, nc.sync, nc.sync]
                        for blk in range(4):
                            lo, n = 30 * blk, min(30, 100 - 30 * blk)
                            engs[blk].dma_start(
                                out_h[lo:lo + n, flc],
                                outba[32 * blk:32 * blk + n, flc])
                    elif c2 % 2 == 1:
                        fl2 = slice((c2 - 1) * WCH, (c2 + 1) * WCH)
                        for blk in range(4):
                            lo, n = 30 * blk, min(30, 100 - 30 * blk)
                            nc.gpsimd.dma_start(
                                out_h[lo:lo + n, fl2],
                                outba[32 * blk:32 * blk + n, fl2])

    nc.compile()
    _PROGRAM = nc
    return nc


def kernel(seg_feat, conv_weight, ind):
    from concourse.bass_utils import run_bass_kernel_spmd

    in_maps, _ = _prep_inputs(seg_feat, conv_weight, ind)
    nc = build_program()
    res = run_bass_kernel_spmd(nc, in_maps, list(range(NCORE)))
    out = np.empty((B, K, HW), dtype=np.float32)
    for core in range(NCORE):
        b = core // 2
        lo = (core % 2) * LC
        out[b, :, lo:lo + LC] = res.results[core]["out_shard"]
    return out.reshape(B, K, H, Wd)
